# revision 22
# baseline (speedup 1.0000x reference)
"""GQA attention (B=2, S=2048, D=2048, 32 q-heads / 8 kv-heads, hd=64),
tensor-parallel over the 8 kv-head groups on 8 NeuronCores.

Per-core math (core c owns kv head c and q heads 4c..4c+3):
  qT = (wq_c @ x.T), kT/vT likewise; RoPE via elementwise muls plus a
  constant pair-swap matmul R; scoresT[sk,sq] = k_rot.T-layout matmul;
  ET = exp(scoresT/8) with causal zeroing on diagonal tiles; out_pvT and
  the softmax denominator come from one matmul against [V | ones];
  partial = attnT.T @ woT_c accumulated in fp32 and summed on host.

Everything on device lives in transposed [feature, seq] layout so no
activation transposes are needed (V is re-transposed per 128-tile via
the PE's transpose mode).
"""

from contextlib import ExitStack

import ml_dtypes
import numpy as np

import concourse.bass as bass
import concourse.tile as tile
from concourse import bacc, mybir
from concourse import bass_utils
from concourse.bass_interp import get_hw_module

BF16 = mybir.dt.bfloat16
F32 = mybir.dt.float32

N_CORES = 8
B, S, DIM = 2, 2048, 2048
NH, NKV, HD = 32, 8, 64          # global heads
NHC = NH // N_CORES              # q heads per core = 4
QD = NHC * HD                    # per-core q out dim = 256
ST = B * S                       # total tokens = 4096
KT = DIM // 128                  # contraction k-tiles = 16
SQT = 512                        # sq tile (matmul free dim)
SKT = 128                        # sk tile (partition dim)
NSQ = S // SQT                   # sq tiles per batch = 4
NSK = S // SKT                   # sk tiles per batch = 16

_CACHE: dict = {}


def _build():
    if "nc" in _CACHE:
        return _CACHE["nc"]
    nc = bacc.Bacc(
        "TRN2",
        target_bir_lowering=False,
        debug=False,
        enable_asserts=False,
        num_devices=N_CORES,
    )
    xT = nc.dram_tensor("xt", [DIM, ST], BF16, kind="ExternalInput").ap()
    wqT = nc.dram_tensor("wqt", [DIM, QD], BF16, kind="ExternalInput").ap()
    wkvT = nc.dram_tensor("wkvt", [DIM, 2 * HD], BF16, kind="ExternalInput").ap()
    woT = nc.dram_tensor("wot", [QD, DIM], BF16, kind="ExternalInput").ap()
    cosE = nc.dram_tensor("cose", [128, ST], BF16, kind="ExternalInput").ap()
    sinE = nc.dram_tensor("sine", [128, ST], BF16, kind="ExternalInput").ap()
    r2t = nc.dram_tensor("r2t", [128, 128], BF16, kind="ExternalInput").ap()
    ident = nc.dram_tensor("ident", [64, 64], BF16, kind="ExternalInput").ap()
    out = nc.dram_tensor("out", [ST, DIM], BF16, kind="ExternalOutput").ap()

    with tile.TileContext(nc) as tc, ExitStack() as ctx:
        pers = ctx.enter_context(tc.tile_pool(name="pers", bufs=1))

        # -- persistent SBUF tensors ------------------------------------
        wq_sb = pers.tile([128, KT * QD], BF16, tag="wq")
        wkv_sb = pers.tile([128, KT * 2 * HD], BF16, tag="wkv")
        wo_sb = [pers.tile([128, DIM], BF16, tag=f"wo{j}", name=f"wo{j}") for j in range(2)]
        cos_sb = pers.tile([128, ST], BF16, tag="cos")
        sin_sb = pers.tile([128, ST], BF16, tag="sin")
        r2t_sb = pers.tile([128, 128], BF16, tag="r2t")
        id_sb = pers.tile([64, 64], BF16, tag="ident")
        qrot = [pers.tile([128, ST], BF16, tag=f"qrot{t}", name=f"qrot{t}") for t in range(2)]
        krot = pers.tile([128, ST], BF16, tag="krot")  # k_rot duplicated in both halves
        vaug = pers.tile([128, B * NSK * 65], BF16, tag="vaug")
        attnT = [pers.tile([128, ST], BF16, tag=f"attnT{t}", name=f"attnT{t}") for t in range(2)]

        nc.sync.dma_start(
            wq_sb.rearrange("p (t d) -> p t d", t=KT), wqT.rearrange("(t p) d -> p t d", p=128)
        )
        nc.sync.dma_start(
            wkv_sb.rearrange("p (t d) -> p t d", t=KT), wkvT.rearrange("(t p) d -> p t d", p=128)
        )
        for j in range(2):
            nc.sync.dma_start(wo_sb[j][:], woT[j * 128:(j + 1) * 128, :])
        nc.sync.dma_start(cos_sb[:], cosE[:])
        nc.sync.dma_start(sin_sb[:], sinE[:])
        nc.sync.dma_start(r2t_sb[:], r2t[:])
        nc.sync.dma_start(id_sb[:], ident[:])
        # ones column of V_aug (col 64 of each 65-wide block)
        nc.gpsimd.memset(vaug[:, 64::65], 1.0)

        # -- unified pools (8 PSUM banks shared across all phases) -------
        with tc.tile_pool(name="xt", bufs=12) as xp, \
             tc.tile_pool(name="stage", bufs=4) as sp, \
             tc.tile_pool(name="et", bufs=6) as ep, \
             tc.tile_pool(name="misc", bufs=3) as mp, \
             tc.tile_pool(name="wout", bufs=4) as woutp, \
             tc.tile_pool(name="ps8", bufs=1, space="PSUM") as pool8:

            def emit_a(st):
                ss = bass.ts(st, SQT)
                psq = [pool8.tile([128, SQT], F32, tag="psq", name="psq",
                                  bufs=2) for _ in range(2)]
                pskv = pool8.tile([128, SQT], F32, tag="pskv", bufs=1)
                for kt in range(KT):
                    xt_t = xp.tile([128, SQT], BF16, name="xt_t")
                    nc.scalar.dma_start(xt_t[:], xT[kt * 128:(kt + 1) * 128, ss])
                    for dt in range(2):
                        nc.tensor.matmul(
                            psq[dt][:],
                            wq_sb[:, kt * QD + dt * 128: kt * QD + (dt + 1) * 128],
                            xt_t[:],
                            start=(kt == 0),
                            stop=(kt == KT - 1),
                        )
                    nc.tensor.matmul(
                        pskv[:],
                        wkv_sb[:, kt * 128:(kt + 1) * 128],
                        xt_t[:],
                        start=(kt == 0),
                        stop=(kt == KT - 1),
                    )
                # rope on q (2 tiles of 128 = 4 heads)
                for dt in range(2):
                    qsb = sp.tile([128, SQT], BF16, tag="qsb", name="qsb")
                    nc.vector.tensor_copy(qsb[:], psq[dt][:])
                    pr = pool8.tile([128, SQT], F32, tag="pr", name="pr", bufs=1)
                    nc.tensor.matmul(pr[:], r2t_sb[:], qsb[:])
                    t1 = sp.tile([128, SQT], BF16, tag="t1", name="t1")
                    nc.vector.tensor_mul(t1[:], qsb[:], cos_sb[:, ss])
                    t2 = sp.tile([128, SQT], BF16, tag="t2", name="t2")
                    nc.vector.tensor_mul(t2[:], pr[:], sin_sb[:, ss])
                    nc.vector.tensor_add(qrot[dt][:, ss], t1[:], t2[:])
                # rope on k (rows 0:64 of kv psum)
                ksb = sp.tile([64, SQT], BF16, tag="ksb", name="ksb")
                nc.vector.tensor_copy(ksb[:], pskv[0:64, :])
                prk_t = pool8.tile([128, SQT], F32, tag="pr", name="prk_t", bufs=1)
                prk = prk_t[0:64, :]
                nc.tensor.matmul(prk[:], r2t_sb[0:64, 0:64], ksb[:])
                t1k = sp.tile([64, SQT], BF16, tag="t1k", name="t1k")
                nc.vector.tensor_mul(t1k[:], ksb[:], cos_sb[0:64, ss])
                t2k = sp.tile([64, SQT], BF16, tag="t2k", name="t2k")
                nc.vector.tensor_mul(t2k[:], prk[:], sin_sb[0:64, ss])
                nc.vector.tensor_add(krot[0:64, ss], t1k[:], t2k[:])
                nc.sync.dma_start(krot[64:128, ss], krot[0:64, ss])
                # V: transpose [64, 128] chunks -> vaug [128, 64] blocks
                vsb = sp.tile([64, SQT], BF16, tag="vsb", name="vsb")
                nc.vector.tensor_copy(vsb[:], pskv[64:128, :])
                for c in range(SQT // 128):
                    j = st * 4 + c  # global sk tile index
                    pt = pool8.tile([128, 64], BF16, tag="pr", name="pt", bufs=1)
                    nc.tensor.transpose(
                        pt[:], vsb[:, c * 128:(c + 1) * 128], id_sb[:]
                    )
                    nc.vector.tensor_copy(vaug[:, j * 65: j * 65 + 64], pt[:])

            def emit_b(b, sqt):
                for dt in range(2):  # head pair (hp=0,1 packed in PE halves)
                    sq0 = b * S + sqt * SQT
                    po = [pool8.tile([65, SQT], F32, tag="po",
                                     name=f"po{hp}", bufs=2) for hp in range(2)]
                    n_sk = 4 * (sqt + 1)
                    for j in range(n_sk):
                        sk0 = b * S + j * SKT
                        d = j - 4 * sqt
                        off = max(0, 128 * d)  # causally dead columns
                        w = SQT - off
                        ps2 = pool8.tile([128, 2 * SQT], F32, tag="ps",
                                         name="ps2", bufs=1)
                        for hp in range(2):
                            hs = slice(hp * 64, (hp + 1) * 64)
                            nc.tensor.matmul(
                                ps2[:, hp * SQT + off:(hp + 1) * SQT],
                                krot[hs, sk0:sk0 + SKT],
                                qrot[dt][hs, sq0 + off:sq0 + SQT],
                                tile_position=(hp * 64, 0),
                            )
                        et2 = ep.tile([128, 2 * SQT], BF16, tag="et2",
                                      name="et2")
                        # one exp over both heads' scores (strided view
                        # skips the causally-dead columns of each half)
                        ps2v = ps2.rearrange("p (h q) -> p h q", h=2)[:, :, off:SQT]
                        et2v = et2.rearrange("p (h q) -> p h q", h=2)[:, :, off:SQT]
                        nc.scalar.activation(
                            et2v, ps2v,
                            mybir.ActivationFunctionType.Exp,
                            scale=0.125,
                        )
                        if d >= 0:  # diagonal tile: zero sk > sq
                            nc.gpsimd.affine_select(
                                out=et2v,
                                in_=et2v,
                                compare_op=mybir.AluOpType.is_ge,
                                fill=0.0,
                                base=0,
                                channel_multiplier=-1,
                                pattern=[[0, 2], [1, w]],
                            )
                        jj = b * NSK + j
                        for hp in range(2):
                            nc.tensor.matmul(
                                po[hp][:, off:SQT],
                                vaug[:, jj * 65:(jj + 1) * 65],
                                et2[:, hp * SQT + off:(hp + 1) * SQT],
                                start=(j == 0),
                                stop=(j == n_sk - 1),
                            )
                    for hp in range(2):
                        den = mp.tile([1, SQT], F32, tag="den", name="den")
                        nc.vector.tensor_copy(den[:], po[hp][64:65, :])
                        recip = mp.tile([1, SQT], F32, tag="recip", name="recip")
                        nc.vector.reciprocal_approx_fast(recip[:], den[:])
                        bc = mp.tile([64, SQT], F32, tag="bc", name="bc")
                        nc.gpsimd.partition_broadcast(bc[:], recip[:])
                        nc.vector.tensor_mul(
                            attnT[dt][hp * 64:(hp + 1) * 64, sq0:sq0 + SQT],
                            po[hp][0:64, :],
                            bc[:],
                        )

            def emit_c(b, sqt):
                for sti in range(SQT // 128):
                    st = (b * S + sqt * SQT) // 128 + sti
                    for ot in range(DIM // SQT):
                        pw = pool8.tile([128, SQT], F32, tag="psq",
                                        name="pw", bufs=2)
                        for jt in range(2):
                            nc.tensor.matmul(
                                pw[:],
                                attnT[jt][:, st * 128:(st + 1) * 128],
                                wo_sb[jt][:, ot * SQT:(ot + 1) * SQT],
                                start=(jt == 0),
                                stop=(jt == 1),
                            )
                        osb = woutp.tile([128, SQT], BF16, tag="osb", name="osb")
                        if (sti + ot) % 2 == 0:
                            nc.vector.tensor_copy(osb[:], pw[:])
                        else:
                            nc.scalar.copy(osb[:], pw[:])
                        nc.sync.dma_start(
                            out[st * 128:(st + 1) * 128,
                                ot * SQT:(ot + 1) * SQT],
                            osb[:],
                        )

            # interleaved schedule: B(b, sqt) needs k/v through A(st); C
            # needs B of its region; C shares the "psq" psum tag with A so
            # all C blocks come after the last A block.
            emit_a(0)
            emit_a(1); emit_b(0, 0)
            emit_a(2); emit_b(0, 1)
            emit_a(3); emit_b(0, 2)
            emit_a(4); emit_b(0, 3)
            emit_a(5); emit_b(1, 0)
            emit_a(6); emit_b(1, 1)
            emit_a(7)
            emit_b(1, 2); emit_c(0, 0); emit_c(0, 1)
            emit_b(1, 3); emit_c(0, 2); emit_c(0, 3)
            emit_c(1, 0); emit_c(1, 1); emit_c(1, 2); emit_c(1, 3)

    nc.compile()
    nc.m = get_hw_module(nc.m)
    _CACHE["nc"] = nc
    return nc


def _prep_inputs(x, freqs_cos, freqs_sin, wq, wk, wv, wo):
    bf = ml_dtypes.bfloat16
    xT = np.ascontiguousarray(x.reshape(ST, DIM).T).astype(bf)
    # expanded rope tables in [feature, seq] layout, tiled over 2 head rows
    cos64 = np.repeat(freqs_cos.T, 2, axis=0)        # [64, S]
    sin64 = np.repeat(freqs_sin.T, 2, axis=0)
    cosE = np.tile(np.tile(cos64, (2, 1)), (1, B)).astype(bf)  # [128, ST]
    sinE = np.tile(np.tile(sin64, (2, 1)), (1, B)).astype(bf)
    # pair-swap matrix R (64x64), block-diagonal doubled, transposed
    R = np.zeros((64, 64), np.float32)
    for i in range(32):
        R[2 * i, 2 * i + 1] = -1.0
        R[2 * i + 1, 2 * i] = 1.0
    R2 = np.zeros((128, 128), np.float32)
    R2[:64, :64] = R
    R2[64:, 64:] = R
    r2t = np.ascontiguousarray(R2.T).astype(bf)
    ident = np.eye(64, dtype=np.float32).astype(bf)

    in_maps = []
    for c in range(N_CORES):
        wq_c = wq[c * QD:(c + 1) * QD, :]
        wk_c = wk[c * HD:(c + 1) * HD, :]
        wv_c = wv[c * HD:(c + 1) * HD, :]
        wkv_c = np.concatenate([wk_c, wv_c], axis=0)   # [128, DIM]
        wo_c = wo[:, c * QD:(c + 1) * QD]              # [DIM, 256]
        in_maps.append({
            "xt": xT,
            "wqt": np.ascontiguousarray(wq_c.T).astype(bf),
            "wkvt": np.ascontiguousarray(wkv_c.T).astype(bf),
            "wot": np.ascontiguousarray(wo_c.T).astype(bf),
            "cose": cosE,
            "sine": sinE,
            "r2t": r2t,
            "ident": ident,
        })
    return in_maps


def kernel(x, freqs_cos, freqs_sin, wq, wk, wv, wo, _trace=False, _trace_kwargs=None):
    x = np.asarray(x, dtype=np.float32)
    freqs_cos = np.asarray(freqs_cos, dtype=np.float32)
    freqs_sin = np.asarray(freqs_sin, dtype=np.float32)
    wq = np.asarray(wq, dtype=np.float32)
    wk = np.asarray(wk, dtype=np.float32)
    wv = np.asarray(wv, dtype=np.float32)
    wo = np.asarray(wo, dtype=np.float32)

    nc = _build()
    in_maps = _prep_inputs(x, freqs_cos, freqs_sin, wq, wk, wv, wo)
    kwargs = dict(_trace_kwargs or {})
    res = bass_utils.run_bass_kernel_spmd(
        nc, in_maps, core_ids=list(range(N_CORES)), trace=_trace, **kwargs
    )
    _CACHE["last_result"] = res
    acc = res.results[0]["out"].astype(np.float32)
    for c in range(1, N_CORES):
        acc += res.results[c]["out"].astype(np.float32)
    return acc.reshape(B, S, DIM)


# revision 23
# speedup vs baseline: 1.0211x; 1.0211x over previous
"""GQA attention (B=2, S=2048, D=2048, 32 q-heads / 8 kv-heads, hd=64),
tensor-parallel over the 8 kv-head groups on 8 NeuronCores.

Per-core math (core c owns kv head c and q heads 4c..4c+3):
  qT = (wq_c @ x.T), kT/vT likewise; RoPE via elementwise muls plus a
  constant pair-swap matmul R; scoresT[sk,sq] = k_rot.T-layout matmul;
  ET = exp(scoresT/8) with causal zeroing on diagonal tiles; out_pvT and
  the softmax denominator come from one matmul against [V | ones];
  partial = attnT.T @ woT_c accumulated in fp32 and summed on host.

Everything on device lives in transposed [feature, seq] layout so no
activation transposes are needed (V is re-transposed per 128-tile via
the PE's transpose mode).
"""

from contextlib import ExitStack

import ml_dtypes
import numpy as np

import concourse.bass as bass
import concourse.tile as tile
from concourse import bacc, mybir
from concourse import bass_utils
from concourse.bass_interp import get_hw_module

BF16 = mybir.dt.bfloat16
F32 = mybir.dt.float32

N_CORES = 8
B, S, DIM = 2, 2048, 2048
NH, NKV, HD = 32, 8, 64          # global heads
NHC = NH // N_CORES              # q heads per core = 4
QD = NHC * HD                    # per-core q out dim = 256
ST = B * S                       # total tokens = 4096
KT = DIM // 128                  # contraction k-tiles = 16
SQT = 512                        # sq tile (matmul free dim)
SKT = 128                        # sk tile (partition dim)
NSQ = S // SQT                   # sq tiles per batch = 4
NSK = S // SKT                   # sk tiles per batch = 16

_CACHE: dict = {}


def _build():
    if "nc" in _CACHE:
        return _CACHE["nc"]
    nc = bacc.Bacc(
        "TRN2",
        target_bir_lowering=False,
        debug=False,
        enable_asserts=False,
        num_devices=N_CORES,
    )
    xT = nc.dram_tensor("xt", [DIM, ST], BF16, kind="ExternalInput").ap()
    wqT = nc.dram_tensor("wqt", [DIM, QD], BF16, kind="ExternalInput").ap()
    wkvT = nc.dram_tensor("wkvt", [DIM, 2 * HD], BF16, kind="ExternalInput").ap()
    woT = nc.dram_tensor("wot", [QD, DIM], BF16, kind="ExternalInput").ap()
    cosE = nc.dram_tensor("cose", [128, ST], BF16, kind="ExternalInput").ap()
    sinE = nc.dram_tensor("sine", [128, ST], BF16, kind="ExternalInput").ap()
    r2t = nc.dram_tensor("r2t", [128, 128], BF16, kind="ExternalInput").ap()
    ident = nc.dram_tensor("ident", [64, 64], BF16, kind="ExternalInput").ap()
    out = nc.dram_tensor("out", [ST, DIM], BF16, kind="ExternalOutput").ap()

    with tile.TileContext(nc) as tc, ExitStack() as ctx:
        pers = ctx.enter_context(tc.tile_pool(name="pers", bufs=1))

        # -- persistent SBUF tensors ------------------------------------
        wq_sb = pers.tile([128, KT * QD], BF16, tag="wq")
        wkv_sb = pers.tile([128, KT * 2 * HD], BF16, tag="wkv")
        wo_sb = [pers.tile([128, DIM], BF16, tag=f"wo{j}", name=f"wo{j}") for j in range(2)]
        cos_sb = pers.tile([128, ST], BF16, tag="cos")
        sin_sb = pers.tile([128, ST], BF16, tag="sin")
        r2t_sb = pers.tile([128, 128], BF16, tag="r2t")
        id_sb = pers.tile([64, 64], BF16, tag="ident")
        qrot = [pers.tile([128, ST], BF16, tag=f"qrot{t}", name=f"qrot{t}") for t in range(2)]
        krot = pers.tile([128, ST], BF16, tag="krot")  # k_rot duplicated in both halves
        vaug = pers.tile([128, B * NSK * 65], BF16, tag="vaug")
        attnT = [pers.tile([128, ST], BF16, tag=f"attnT{t}", name=f"attnT{t}") for t in range(2)]

        nc.sync.dma_start(
            wq_sb.rearrange("p (t d) -> p t d", t=KT), wqT.rearrange("(t p) d -> p t d", p=128)
        )
        nc.sync.dma_start(
            wkv_sb.rearrange("p (t d) -> p t d", t=KT), wkvT.rearrange("(t p) d -> p t d", p=128)
        )
        for j in range(2):
            nc.sync.dma_start(wo_sb[j][:], woT[j * 128:(j + 1) * 128, :])
        nc.sync.dma_start(cos_sb[:], cosE[:])
        nc.sync.dma_start(sin_sb[:], sinE[:])
        nc.sync.dma_start(r2t_sb[:], r2t[:])
        nc.sync.dma_start(id_sb[:], ident[:])
        # ones column of V_aug (col 64 of each 65-wide block)
        nc.gpsimd.memset(vaug[:, 64::65], 1.0)

        # -- unified pools (8 PSUM banks shared across all phases) -------
        with tc.tile_pool(name="xt", bufs=12) as xp, \
             tc.tile_pool(name="stage", bufs=4) as sp, \
             tc.tile_pool(name="et", bufs=6) as ep, \
             tc.tile_pool(name="misc", bufs=3) as mp, \
             tc.tile_pool(name="wout", bufs=4) as woutp, \
             tc.tile_pool(name="ps8", bufs=1, space="PSUM") as pool8:

            def emit_a(st):
                ss = bass.ts(st, SQT)
                psq = [pool8.tile([128, SQT], F32, tag="psq", name="psq",
                                  bufs=2) for _ in range(2)]
                pskv = pool8.tile([128, SQT], F32, tag="pskv", bufs=1)
                for kt in range(KT):
                    xt_t = xp.tile([128, SQT], BF16, name="xt_t")
                    nc.scalar.dma_start(xt_t[:], xT[kt * 128:(kt + 1) * 128, ss])
                    for dt in range(2):
                        nc.tensor.matmul(
                            psq[dt][:],
                            wq_sb[:, kt * QD + dt * 128: kt * QD + (dt + 1) * 128],
                            xt_t[:],
                            start=(kt == 0),
                            stop=(kt == KT - 1),
                        )
                    nc.tensor.matmul(
                        pskv[:],
                        wkv_sb[:, kt * 128:(kt + 1) * 128],
                        xt_t[:],
                        start=(kt == 0),
                        stop=(kt == KT - 1),
                    )
                # rope on q (2 tiles of 128 = 4 heads)
                for dt in range(2):
                    qsb = sp.tile([128, SQT], BF16, tag="qsb", name="qsb")
                    nc.vector.tensor_copy(qsb[:], psq[dt][:])
                    pr = pool8.tile([128, SQT], F32, tag="pr", name="pr", bufs=1)
                    nc.tensor.matmul(pr[:], r2t_sb[:], qsb[:])
                    t1 = sp.tile([128, SQT], BF16, tag="t1", name="t1")
                    nc.vector.tensor_mul(t1[:], qsb[:], cos_sb[:, ss])
                    t2 = sp.tile([128, SQT], BF16, tag="t2", name="t2")
                    nc.vector.tensor_mul(t2[:], pr[:], sin_sb[:, ss])
                    nc.vector.tensor_add(qrot[dt][:, ss], t1[:], t2[:])
                # rope on k (rows 0:64 of kv psum)
                ksb = sp.tile([64, SQT], BF16, tag="ksb", name="ksb")
                nc.vector.tensor_copy(ksb[:], pskv[0:64, :])
                prk_t = pool8.tile([128, SQT], F32, tag="pr", name="prk_t", bufs=1)
                prk = prk_t[0:64, :]
                nc.tensor.matmul(prk[:], r2t_sb[0:64, 0:64], ksb[:])
                t1k = sp.tile([64, SQT], BF16, tag="t1k", name="t1k")
                nc.vector.tensor_mul(t1k[:], ksb[:], cos_sb[0:64, ss])
                t2k = sp.tile([64, SQT], BF16, tag="t2k", name="t2k")
                nc.vector.tensor_mul(t2k[:], prk[:], sin_sb[0:64, ss])
                nc.vector.tensor_add(krot[0:64, ss], t1k[:], t2k[:])
                nc.sync.dma_start(krot[64:128, ss], krot[0:64, ss])
                # V: transpose [64, 128] chunks -> vaug [128, 64] blocks
                vsb = sp.tile([64, SQT], BF16, tag="vsb", name="vsb")
                nc.vector.tensor_copy(vsb[:], pskv[64:128, :])
                for c in range(SQT // 128):
                    j = st * 4 + c  # global sk tile index
                    pt = pool8.tile([128, 64], BF16, tag="pr", name="pt", bufs=1)
                    nc.tensor.transpose(
                        pt[:], vsb[:, c * 128:(c + 1) * 128], id_sb[:]
                    )
                    nc.vector.tensor_copy(vaug[:, j * 65: j * 65 + 64], pt[:])

            def emit_b(b, sqt):
                for dt in range(2):  # head pair (hp=0,1 packed in PE halves)
                    sq0 = b * S + sqt * SQT
                    po = [pool8.tile([65, SQT], F32, tag="po",
                                     name=f"po{hp}", bufs=2) for hp in range(2)]
                    n_sk = 4 * (sqt + 1)
                    for j in range(n_sk):
                        sk0 = b * S + j * SKT
                        d = j - 4 * sqt
                        off = max(0, 128 * d)  # causally dead columns
                        w = SQT - off
                        pss = []
                        for hp in range(2):
                            hs = slice(hp * 64, (hp + 1) * 64)
                            ps = pool8.tile([128, SQT], F32, tag="ps",
                                            name="ps", bufs=2)
                            nc.tensor.matmul(
                                ps[:, off:SQT],
                                krot[hs, sk0:sk0 + SKT],
                                qrot[dt][hs, sq0 + off:sq0 + SQT],
                                tile_position=(hp * 64, 0),
                            )
                            pss.append(ps)
                        ets = []
                        for hp in range(2):
                            et = ep.tile([128, SQT], BF16, tag=f"et{hp}",
                                         name=f"et{hp}")
                            nc.scalar.activation(
                                et[:, off:SQT], pss[hp][:, off:SQT],
                                mybir.ActivationFunctionType.Exp,
                                scale=0.125,
                            )
                            if d >= 0:  # diagonal tile: zero sk > sq
                                nc.gpsimd.affine_select(
                                    out=et[:, off:SQT],
                                    in_=et[:, off:SQT],
                                    compare_op=mybir.AluOpType.is_ge,
                                    fill=0.0,
                                    base=0,
                                    channel_multiplier=-1,
                                    pattern=[[1, w]],
                                )
                            ets.append(et)
                        jj = b * NSK + j
                        for hp in range(2):
                            nc.tensor.matmul(
                                po[hp][:, off:SQT],
                                vaug[:, jj * 65:(jj + 1) * 65],
                                ets[hp][:, off:SQT],
                                start=(j == 0),
                                stop=(j == n_sk - 1),
                            )
                    for hp in range(2):
                        den = mp.tile([1, SQT], F32, tag="den", name="den")
                        nc.vector.tensor_copy(den[:], po[hp][64:65, :])
                        recip = mp.tile([1, SQT], F32, tag="recip", name="recip")
                        nc.vector.reciprocal_approx_fast(recip[:], den[:])
                        bc = mp.tile([64, SQT], F32, tag="bc", name="bc")
                        nc.gpsimd.partition_broadcast(bc[:], recip[:])
                        nc.vector.tensor_mul(
                            attnT[dt][hp * 64:(hp + 1) * 64, sq0:sq0 + SQT],
                            po[hp][0:64, :],
                            bc[:],
                        )

            def emit_c(b, sqt):
                for sti in range(SQT // 128):
                    st = (b * S + sqt * SQT) // 128 + sti
                    for ot in range(DIM // SQT):
                        pw = pool8.tile([128, SQT], F32, tag="psq",
                                        name="pw", bufs=2)
                        for jt in range(2):
                            nc.tensor.matmul(
                                pw[:],
                                attnT[jt][:, st * 128:(st + 1) * 128],
                                wo_sb[jt][:, ot * SQT:(ot + 1) * SQT],
                                start=(jt == 0),
                                stop=(jt == 1),
                            )
                        osb = woutp.tile([128, SQT], BF16, tag="osb", name="osb")
                        if (sti + ot) % 2 == 0:
                            nc.vector.tensor_copy(osb[:], pw[:])
                        else:
                            nc.scalar.copy(osb[:], pw[:])
                        nc.sync.dma_start(
                            out[st * 128:(st + 1) * 128,
                                ot * SQT:(ot + 1) * SQT],
                            osb[:],
                        )

            # interleaved schedule: B(b, sqt) needs k/v through A(st); C
            # needs B of its region; C shares the "psq" psum tag with A so
            # all C blocks come after the last A block.
            emit_a(0)
            emit_a(1); emit_b(0, 0)
            emit_a(2); emit_b(0, 1)
            emit_a(3); emit_b(0, 2)
            emit_a(4); emit_b(0, 3)
            emit_a(5); emit_b(1, 0)
            emit_a(6); emit_b(1, 1)
            emit_a(7)
            emit_b(1, 2); emit_c(0, 0); emit_c(0, 1)
            emit_b(1, 3); emit_c(0, 2); emit_c(0, 3)
            emit_c(1, 0); emit_c(1, 1); emit_c(1, 2); emit_c(1, 3)

    nc.compile()
    nc.m = get_hw_module(nc.m)
    _CACHE["nc"] = nc
    return nc


def _prep_inputs(x, freqs_cos, freqs_sin, wq, wk, wv, wo):
    bf = ml_dtypes.bfloat16
    xT = np.ascontiguousarray(x.reshape(ST, DIM).T).astype(bf)
    # expanded rope tables in [feature, seq] layout, tiled over 2 head rows
    cos64 = np.repeat(freqs_cos.T, 2, axis=0)        # [64, S]
    sin64 = np.repeat(freqs_sin.T, 2, axis=0)
    cosE = np.tile(np.tile(cos64, (2, 1)), (1, B)).astype(bf)  # [128, ST]
    sinE = np.tile(np.tile(sin64, (2, 1)), (1, B)).astype(bf)
    # pair-swap matrix R (64x64), block-diagonal doubled, transposed
    R = np.zeros((64, 64), np.float32)
    for i in range(32):
        R[2 * i, 2 * i + 1] = -1.0
        R[2 * i + 1, 2 * i] = 1.0
    R2 = np.zeros((128, 128), np.float32)
    R2[:64, :64] = R
    R2[64:, 64:] = R
    r2t = np.ascontiguousarray(R2.T).astype(bf)
    ident = np.eye(64, dtype=np.float32).astype(bf)

    in_maps = []
    for c in range(N_CORES):
        wq_c = wq[c * QD:(c + 1) * QD, :]
        wk_c = wk[c * HD:(c + 1) * HD, :]
        wv_c = wv[c * HD:(c + 1) * HD, :]
        wkv_c = np.concatenate([wk_c, wv_c], axis=0)   # [128, DIM]
        wo_c = wo[:, c * QD:(c + 1) * QD]              # [DIM, 256]
        in_maps.append({
            "xt": xT,
            "wqt": np.ascontiguousarray(wq_c.T).astype(bf),
            "wkvt": np.ascontiguousarray(wkv_c.T).astype(bf),
            "wot": np.ascontiguousarray(wo_c.T).astype(bf),
            "cose": cosE,
            "sine": sinE,
            "r2t": r2t,
            "ident": ident,
        })
    return in_maps


def kernel(x, freqs_cos, freqs_sin, wq, wk, wv, wo, _trace=False, _trace_kwargs=None):
    x = np.asarray(x, dtype=np.float32)
    freqs_cos = np.asarray(freqs_cos, dtype=np.float32)
    freqs_sin = np.asarray(freqs_sin, dtype=np.float32)
    wq = np.asarray(wq, dtype=np.float32)
    wk = np.asarray(wk, dtype=np.float32)
    wv = np.asarray(wv, dtype=np.float32)
    wo = np.asarray(wo, dtype=np.float32)

    nc = _build()
    in_maps = _prep_inputs(x, freqs_cos, freqs_sin, wq, wk, wv, wo)
    kwargs = dict(_trace_kwargs or {})
    res = bass_utils.run_bass_kernel_spmd(
        nc, in_maps, core_ids=list(range(N_CORES)), trace=_trace, **kwargs
    )
    _CACHE["last_result"] = res
    acc = res.results[0]["out"].astype(np.float32)
    for c in range(1, N_CORES):
        acc += res.results[c]["out"].astype(np.float32)
    return acc.reshape(B, S, DIM)


# revision 24
# speedup vs baseline: 1.0596x; 1.0378x over previous
"""GQA attention (B=2, S=2048, D=2048, 32 q-heads / 8 kv-heads, hd=64),
tensor-parallel over the 8 kv-head groups on 8 NeuronCores.

Per-core math (core c owns kv head c and q heads 4c..4c+3):
  qT = (wq_c @ x.T), kT/vT likewise; RoPE via elementwise muls plus a
  constant pair-swap matmul R; scoresT[sk,sq] = k_rot.T-layout matmul;
  ET = exp(scoresT/8) with causal zeroing on diagonal tiles; out_pvT and
  the softmax denominator come from one matmul against [V | ones];
  partial = attnT.T @ woT_c accumulated in fp32 and summed on host.

Everything on device lives in transposed [feature, seq] layout so no
activation transposes are needed (V is re-transposed per 128-tile via
the PE's transpose mode).
"""

from contextlib import ExitStack

import ml_dtypes
import numpy as np

import concourse.bass as bass
import concourse.tile as tile
from concourse import bacc, mybir
from concourse import bass_utils
from concourse.bass_interp import get_hw_module

BF16 = mybir.dt.bfloat16
F32 = mybir.dt.float32

N_CORES = 8
B, S, DIM = 2, 2048, 2048
NH, NKV, HD = 32, 8, 64          # global heads
NHC = NH // N_CORES              # q heads per core = 4
QD = NHC * HD                    # per-core q out dim = 256
ST = B * S                       # total tokens = 4096
KT = DIM // 128                  # contraction k-tiles = 16
SQT = 512                        # sq tile (matmul free dim)
SKT = 128                        # sk tile (partition dim)
NSQ = S // SQT                   # sq tiles per batch = 4
NSK = S // SKT                   # sk tiles per batch = 16

_CACHE: dict = {}


def _build():
    if "nc" in _CACHE:
        return _CACHE["nc"]
    nc = bacc.Bacc(
        "TRN2",
        target_bir_lowering=False,
        debug=False,
        enable_asserts=False,
        num_devices=N_CORES,
    )
    xT = nc.dram_tensor("xt", [DIM, ST], BF16, kind="ExternalInput").ap()
    wqT = nc.dram_tensor("wqt", [DIM, QD], BF16, kind="ExternalInput").ap()
    wkvT = nc.dram_tensor("wkvt", [DIM, 2 * HD], BF16, kind="ExternalInput").ap()
    woT = nc.dram_tensor("wot", [QD, DIM], BF16, kind="ExternalInput").ap()
    cosE = nc.dram_tensor("cose", [128, ST], BF16, kind="ExternalInput").ap()
    sinE = nc.dram_tensor("sine", [128, ST], BF16, kind="ExternalInput").ap()
    r2t = nc.dram_tensor("r2t", [128, 128], BF16, kind="ExternalInput").ap()
    ident = nc.dram_tensor("ident", [64, 64], BF16, kind="ExternalInput").ap()
    out = nc.dram_tensor("out", [ST, DIM], BF16, kind="ExternalOutput").ap()

    with tile.TileContext(nc) as tc, ExitStack() as ctx:
        pers = ctx.enter_context(tc.tile_pool(name="pers", bufs=1))

        # -- persistent SBUF tensors ------------------------------------
        wq_sb = pers.tile([128, KT * QD], BF16, tag="wq")
        wkv_sb = pers.tile([128, KT * 2 * HD], BF16, tag="wkv")
        wo_sb = [pers.tile([128, DIM], BF16, tag=f"wo{j}", name=f"wo{j}") for j in range(2)]
        cos_sb = pers.tile([128, ST], BF16, tag="cos")
        sin_sb = pers.tile([128, ST], BF16, tag="sin")
        r2t_sb = pers.tile([128, 128], BF16, tag="r2t")
        id_sb = pers.tile([64, 64], BF16, tag="ident")
        qrot = [pers.tile([128, ST], BF16, tag=f"qrot{t}", name=f"qrot{t}") for t in range(2)]
        krot = pers.tile([128, ST], BF16, tag="krot")  # k_rot duplicated in both halves
        vaug = pers.tile([128, B * NSK * 65], BF16, tag="vaug")
        attnT = [pers.tile([128, ST], BF16, tag=f"attnT{t}", name=f"attnT{t}") for t in range(2)]

        nc.sync.dma_start(
            wq_sb.rearrange("p (t d) -> p t d", t=KT), wqT.rearrange("(t p) d -> p t d", p=128)
        )
        nc.sync.dma_start(
            wkv_sb.rearrange("p (t d) -> p t d", t=KT), wkvT.rearrange("(t p) d -> p t d", p=128)
        )
        for j in range(2):
            nc.sync.dma_start(wo_sb[j][:], woT[j * 128:(j + 1) * 128, :])
        nc.sync.dma_start(cos_sb[:], cosE[:])
        nc.sync.dma_start(sin_sb[:], sinE[:])
        nc.sync.dma_start(r2t_sb[:], r2t[:])
        nc.sync.dma_start(id_sb[:], ident[:])
        # ones column of V_aug (col 64 of each 65-wide block)
        nc.gpsimd.memset(vaug[:, 64::65], 1.0)

        # -- unified pools (8 PSUM banks shared across all phases) -------
        with tc.tile_pool(name="xt", bufs=18) as xp, \
             tc.tile_pool(name="stage", bufs=4) as sp, \
             tc.tile_pool(name="et", bufs=6) as ep, \
             tc.tile_pool(name="misc", bufs=3) as mp, \
             tc.tile_pool(name="wout", bufs=8) as woutp, \
             tc.tile_pool(name="ps8", bufs=1, space="PSUM") as pool8:

            xt_cache = {}

            def emit_a(st):
                ss = bass.ts(st, SQT)
                psq = [pool8.tile([128, SQT], F32, tag="psq", name="psq",
                                  bufs=2) for _ in range(2)]
                pskv = pool8.tile([128, SQT], F32, tag="pskv", bufs=1)
                if st % 2 == 0:
                    xt_cache.clear()
                    for kt in range(KT):
                        t = xp.tile([128, 2 * SQT], BF16, name="xt_t")
                        nc.scalar.dma_start(
                            t[:], xT[kt * 128:(kt + 1) * 128,
                                     st * SQT:(st + 2) * SQT]
                        )
                        xt_cache[kt] = t
                for kt in range(KT):
                    xt_t = xt_cache[kt][:, (st % 2) * SQT:(st % 2 + 1) * SQT]
                    for dt in range(2):
                        nc.tensor.matmul(
                            psq[dt][:],
                            wq_sb[:, kt * QD + dt * 128: kt * QD + (dt + 1) * 128],
                            xt_t[:],
                            start=(kt == 0),
                            stop=(kt == KT - 1),
                        )
                    nc.tensor.matmul(
                        pskv[:],
                        wkv_sb[:, kt * 128:(kt + 1) * 128],
                        xt_t[:],
                        start=(kt == 0),
                        stop=(kt == KT - 1),
                    )
                # rope on q (2 tiles of 128 = 4 heads)
                for dt in range(2):
                    qsb = sp.tile([128, SQT], BF16, tag="qsb", name="qsb")
                    nc.vector.tensor_copy(qsb[:], psq[dt][:])
                    pr = pool8.tile([128, SQT], F32, tag="pr", name="pr", bufs=1)
                    nc.tensor.matmul(pr[:], r2t_sb[:], qsb[:])
                    t1 = sp.tile([128, SQT], BF16, tag="t1", name="t1")
                    nc.vector.tensor_mul(t1[:], qsb[:], cos_sb[:, ss])
                    t2 = sp.tile([128, SQT], BF16, tag="t2", name="t2")
                    nc.vector.tensor_mul(t2[:], pr[:], sin_sb[:, ss])
                    nc.vector.tensor_add(qrot[dt][:, ss], t1[:], t2[:])
                # rope on k (rows 0:64 of kv psum)
                ksb = sp.tile([64, SQT], BF16, tag="ksb", name="ksb")
                nc.vector.tensor_copy(ksb[:], pskv[0:64, :])
                prk_t = pool8.tile([128, SQT], F32, tag="pr", name="prk_t", bufs=1)
                prk = prk_t[0:64, :]
                nc.tensor.matmul(prk[:], r2t_sb[0:64, 0:64], ksb[:])
                t1k = sp.tile([64, SQT], BF16, tag="t1k", name="t1k")
                nc.vector.tensor_mul(t1k[:], ksb[:], cos_sb[0:64, ss])
                t2k = sp.tile([64, SQT], BF16, tag="t2k", name="t2k")
                nc.vector.tensor_mul(t2k[:], prk[:], sin_sb[0:64, ss])
                nc.vector.tensor_add(krot[0:64, ss], t1k[:], t2k[:])
                nc.sync.dma_start(krot[64:128, ss], krot[0:64, ss])
                # V: transpose [64, 128] chunks -> vaug [128, 64] blocks
                vsb = sp.tile([64, SQT], BF16, tag="vsb", name="vsb")
                nc.vector.tensor_copy(vsb[:], pskv[64:128, :])
                for c in range(SQT // 128):
                    j = st * 4 + c  # global sk tile index
                    pt = pool8.tile([128, 64], BF16, tag="pr", name="pt", bufs=1)
                    nc.tensor.transpose(
                        pt[:], vsb[:, c * 128:(c + 1) * 128], id_sb[:]
                    )
                    nc.vector.tensor_copy(vaug[:, j * 65: j * 65 + 64], pt[:])

            def emit_b(b, sqt):
                for dt in range(2):  # head pair (hp=0,1 packed in PE halves)
                    sq0 = b * S + sqt * SQT
                    po = [pool8.tile([65, SQT], F32, tag="po",
                                     name=f"po{hp}", bufs=2) for hp in range(2)]
                    n_sk = 4 * (sqt + 1)
                    for j in range(n_sk):
                        sk0 = b * S + j * SKT
                        d = j - 4 * sqt
                        off = max(0, 128 * d)  # causally dead columns
                        w = SQT - off
                        pss = []
                        for hp in range(2):
                            hs = slice(hp * 64, (hp + 1) * 64)
                            ps = pool8.tile([128, SQT], F32, tag="ps",
                                            name="ps", bufs=2)
                            nc.tensor.matmul(
                                ps[:, off:SQT],
                                krot[hs, sk0:sk0 + SKT],
                                qrot[dt][hs, sq0 + off:sq0 + SQT],
                                tile_position=(hp * 64, 0),
                            )
                            pss.append(ps)
                        ets = []
                        for hp in range(2):
                            et = ep.tile([128, SQT], BF16, tag=f"et{hp}",
                                         name=f"et{hp}")
                            nc.scalar.activation(
                                et[:, off:SQT], pss[hp][:, off:SQT],
                                mybir.ActivationFunctionType.Exp,
                                scale=0.125,
                            )
                            if d >= 0:  # diagonal tile: zero sk > sq
                                nc.gpsimd.affine_select(
                                    out=et[:, off:SQT],
                                    in_=et[:, off:SQT],
                                    compare_op=mybir.AluOpType.is_ge,
                                    fill=0.0,
                                    base=0,
                                    channel_multiplier=-1,
                                    pattern=[[1, w]],
                                )
                            ets.append(et)
                        jj = b * NSK + j
                        for hp in range(2):
                            nc.tensor.matmul(
                                po[hp][:, off:SQT],
                                vaug[:, jj * 65:(jj + 1) * 65],
                                ets[hp][:, off:SQT],
                                start=(j == 0),
                                stop=(j == n_sk - 1),
                            )
                    for hp in range(2):
                        den = mp.tile([1, SQT], F32, tag="den", name="den")
                        nc.vector.tensor_copy(den[:], po[hp][64:65, :])
                        recip = mp.tile([1, SQT], F32, tag="recip", name="recip")
                        nc.vector.reciprocal_approx_fast(recip[:], den[:])
                        bc = mp.tile([64, SQT], F32, tag="bc", name="bc")
                        nc.gpsimd.partition_broadcast(bc[:], recip[:])
                        nc.vector.tensor_mul(
                            attnT[dt][hp * 64:(hp + 1) * 64, sq0:sq0 + SQT],
                            po[hp][0:64, :],
                            bc[:],
                        )

            def emit_c(b, sqt):
                for sti in range(SQT // 128):
                    st = (b * S + sqt * SQT) // 128 + sti
                    for ot in range(DIM // SQT):
                        pw = pool8.tile([128, SQT], F32, tag="psq",
                                        name="pw", bufs=2)
                        for jt in range(2):
                            nc.tensor.matmul(
                                pw[:],
                                attnT[jt][:, st * 128:(st + 1) * 128],
                                wo_sb[jt][:, ot * SQT:(ot + 1) * SQT],
                                start=(jt == 0),
                                stop=(jt == 1),
                            )
                        osb = woutp.tile([128, SQT], BF16, tag="osb", name="osb")
                        if (sti + ot) % 2 == 0:
                            nc.vector.tensor_copy(osb[:], pw[:])
                        else:
                            nc.scalar.copy(osb[:], pw[:])
                        nc.sync.dma_start(
                            out[st * 128:(st + 1) * 128,
                                ot * SQT:(ot + 1) * SQT],
                            osb[:],
                        )

            # interleaved schedule: B(b, sqt) needs k/v through A(st); C
            # needs B of its region; C shares the "psq" psum tag with A so
            # all C blocks come after the last A block.
            emit_a(0)
            emit_a(1); emit_b(0, 0)
            emit_a(2); emit_b(0, 1)
            emit_a(3); emit_b(0, 2)
            emit_a(4); emit_b(0, 3)
            emit_a(5); emit_b(1, 0)
            emit_a(6); emit_b(1, 1)
            emit_a(7)
            emit_b(1, 2); emit_c(0, 0); emit_c(0, 1)
            emit_c(0, 2); emit_c(0, 3); emit_c(1, 0); emit_c(1, 1)
            emit_b(1, 3)
            emit_c(1, 2); emit_c(1, 3)

    nc.compile()
    nc.m = get_hw_module(nc.m)
    _CACHE["nc"] = nc
    return nc


def _prep_inputs(x, freqs_cos, freqs_sin, wq, wk, wv, wo):
    bf = ml_dtypes.bfloat16
    xT = np.ascontiguousarray(x.reshape(ST, DIM).T).astype(bf)
    # expanded rope tables in [feature, seq] layout, tiled over 2 head rows
    cos64 = np.repeat(freqs_cos.T, 2, axis=0)        # [64, S]
    sin64 = np.repeat(freqs_sin.T, 2, axis=0)
    cosE = np.tile(np.tile(cos64, (2, 1)), (1, B)).astype(bf)  # [128, ST]
    sinE = np.tile(np.tile(sin64, (2, 1)), (1, B)).astype(bf)
    # pair-swap matrix R (64x64), block-diagonal doubled, transposed
    R = np.zeros((64, 64), np.float32)
    for i in range(32):
        R[2 * i, 2 * i + 1] = -1.0
        R[2 * i + 1, 2 * i] = 1.0
    R2 = np.zeros((128, 128), np.float32)
    R2[:64, :64] = R
    R2[64:, 64:] = R
    r2t = np.ascontiguousarray(R2.T).astype(bf)
    ident = np.eye(64, dtype=np.float32).astype(bf)

    in_maps = []
    for c in range(N_CORES):
        wq_c = wq[c * QD:(c + 1) * QD, :]
        wk_c = wk[c * HD:(c + 1) * HD, :]
        wv_c = wv[c * HD:(c + 1) * HD, :]
        wkv_c = np.concatenate([wk_c, wv_c], axis=0)   # [128, DIM]
        wo_c = wo[:, c * QD:(c + 1) * QD]              # [DIM, 256]
        in_maps.append({
            "xt": xT,
            "wqt": np.ascontiguousarray(wq_c.T).astype(bf),
            "wkvt": np.ascontiguousarray(wkv_c.T).astype(bf),
            "wot": np.ascontiguousarray(wo_c.T).astype(bf),
            "cose": cosE,
            "sine": sinE,
            "r2t": r2t,
            "ident": ident,
        })
    return in_maps


def kernel(x, freqs_cos, freqs_sin, wq, wk, wv, wo, _trace=False, _trace_kwargs=None):
    x = np.asarray(x, dtype=np.float32)
    freqs_cos = np.asarray(freqs_cos, dtype=np.float32)
    freqs_sin = np.asarray(freqs_sin, dtype=np.float32)
    wq = np.asarray(wq, dtype=np.float32)
    wk = np.asarray(wk, dtype=np.float32)
    wv = np.asarray(wv, dtype=np.float32)
    wo = np.asarray(wo, dtype=np.float32)

    nc = _build()
    in_maps = _prep_inputs(x, freqs_cos, freqs_sin, wq, wk, wv, wo)
    kwargs = dict(_trace_kwargs or {})
    res = bass_utils.run_bass_kernel_spmd(
        nc, in_maps, core_ids=list(range(N_CORES)), trace=_trace, **kwargs
    )
    _CACHE["last_result"] = res
    acc = res.results[0]["out"].astype(np.float32)
    for c in range(1, N_CORES):
        acc += res.results[c]["out"].astype(np.float32)
    return acc.reshape(B, S, DIM)
